# revision 12
# baseline (speedup 1.0000x reference)
"""Trainium2 Bass kernel for nn_Cross_modal_ContrastiveLoss6.

Math: the reference loss only depends on per-class means of the two
modalities (every entry of the N x N distance matrix is determined by the
class pair), so the whole computation reduces to:

  1. raw per-class segment sums R[c,d], T[c,d]  (memory-bound pass over x)
  2. the three 128x128 class Gram matrices P1 = R R^T, P2 = T T^T, P3 = R T^T
  3. tiny 128x128 class-pair loss math with the class counts

Device strategy (8 cores, feature/d-sharded so no cross-core collective is
needed): core k takes columns [256k, 256k+256) of both modal tensors and
computes the full-N segment sums for its d-chunk with one-hot matmuls on
the PE.  The x data is quantized host-side to fp8 e4m3 (measured end-to-end
loss rel-err ~6e-4, well inside the 2e-2 gate) which makes the HBM read
2.1 MiB per core.  Sample blocks are processed in PAIRS with fp8 DoubleRow
matmuls ([128, 2, 128] one-hot stationary x [128, 2, 512] moving, 2 MACs
per cell per cycle) so the PE keeps up with the DMA stream even while the
power manager caps it at half clock during heavy DMA.  One-hot stationaries
are generated on the otherwise idle DVE from an int8 iota row and packed
targets (prepended to x chunk 0, bit-cast in SBUF) via a broadcast-AP
is_equal.  The stream is spread over all three DMA-capable queues
(sync/scalar HWDGE + gpsimd SWDGE).  The segment sums return as bf16; the
host forms the three Grams and does the count scaling + sqrt/relu/weighted
mean (<0.1% of the FLOPs) in float64.
"""

import numpy as np
import ml_dtypes

import concourse.bass as bass
import concourse.mybir as mybir
from concourse.bass_utils import run_bass_kernel_spmd

N = 4096
D = 2048
C = 128
MARGIN = 0.5
NCORES = 8
DCHUNK = D // NCORES          # 256 feature columns per core
P = 128                       # partitions / sample-block size
NB = N // P                   # 32 sample blocks
BWB = 512                     # fp8 bytes per partition per block (x1 256 | x2 256)
CW = C + NB                   # consts bytes per partition (iota row + targets)
# Chunks (in 128-sample blocks; all even so DoubleRow pairs never straddle),
# round-robin over the sync / scalar / gpsimd queues.  Entries on one queue
# stream back-to-back; each entry's semaphore lags its last byte by ~1.4us,
# so chunks are kept moderate for smooth PE gating, and the consts ride in
# front of chunk 0 (no separate tiny transfer).
CHUNKS = [2, 4, 6, 8, 6, 4, 2]
QUEUE = ["sync", "scalar", "sync", "scalar", "gpsimd", "sync", "scalar"]
# PE consumes chunks in expected-semaphore-arrival order (PSUM accumulation
# is additive, so any fixed order is valid).
PE_ORDER = [0, 1, 2, 3, 5, 4, 6]
NCHUNK = len(CHUNKS)
CHUNK_OFF = [sum(CHUNKS[:i]) for i in range(NCHUNK + 1)]
NWARM = 10                    # junk matmuls to ramp the PE clock (HAM gate)

F32 = mybir.dt.float32
BF16 = mybir.dt.bfloat16
I8 = mybir.dt.int8
FP8 = mybir.dt.float8e4
NPBF16 = ml_dtypes.bfloat16
NPFP8 = ml_dtypes.float8_e4m3

_PROGRAM = None


def _build_program() -> bass.Bass:
    nc = bass.Bass()

    # x[:, 0:160]   = consts (int8 bits in the fp8 container): iota row
    #                 (iota[p, c] = c) then targets tgt[p, b] = targets[b*128+p].
    # x[:, 160 + b*512 : 160 + (b+1)*512] = fp8 [x1 | x2] for sample block b.
    x_in = nc.declare_dram_parameter("x", [P, CW + NB * BWB], FP8, isOutput=False)
    # sums[:, 0:256] = R segment sums for this core's d-chunk, [:, 256:512] = T
    sums_out = nc.declare_dram_parameter("sums", [P, 512], BF16, isOutput=True)

    import contextlib

    with contextlib.ExitStack() as stack:
        x_t = stack.enter_context(nc.sbuf_tensor([P, CW + NB * BWB], FP8))
        oh_t = stack.enter_context(nc.sbuf_tensor([P, NB * C], FP8))
        warm_t = stack.enter_context(nc.sbuf_tensor([P, 640], FP8))
        out_t = stack.enter_context(nc.sbuf_tensor([P, 512], BF16))
        psum_acc = stack.enter_context(nc.psum_tensor([P, 512], F32))
        psum_warm = stack.enter_context(nc.psum_tensor([P, 512], F32))

        def sem(name):
            return stack.enter_context(nc.semaphore(name))

        oh_gen = sem("oh_gen")
        x_sems = [sem(f"x_dma_{j}") for j in range(NCHUNK)]
        pe_done = sem("pe_done")
        vec_done = sem("vec_done")
        gp_done = sem("gp_done")
        dma_out = sem("dma_out")

        # Raw-bass semaphores are NOT cleared by the framework preamble; zero
        # them on gpsimd before the Block.  The Block-entry rendezvous keeps
        # every other engine from touching them until the clear has run.
        all_sems = [oh_gen] + x_sems + [pe_done, vec_done, gp_done, dma_out]
        nums = sorted(h.num for h in all_sems)
        assert nums == list(range(nums[0], nums[0] + len(nums))), nums
        nc.gpsimd.sem_clear(range(nums[0], nums[-1] + 1))

        def x_slice(j):
            a = CHUNK_OFF[j] * BWB + (0 if j == 0 else CW)
            b = CHUNK_OFF[j + 1] * BWB + CW
            return slice(a, b)

        def issue_chunks(eng, name):
            for j in range(NCHUNK):
                if QUEUE[j] != name:
                    continue
                sl = x_slice(j)
                eng.dma_start(out=x_t[:, sl], in_=x_in[:, sl]).then_inc(
                    x_sems[j], 16
                )

        with nc.Block(no_gpsimd_drain=True) as block:

            @block.sync
            def _(sync: bass.BassEngine):
                issue_chunks(sync, "sync")
                sync.wait_ge(vec_done, 1)
                sync.dma_start(
                    out=sums_out[:, 0:256], in_=out_t[:, 0:256]
                ).then_inc(dma_out, 16)
                sync.wait_ge(dma_out, 32)

            @block.scalar
            def _(scalar: bass.BassEngine):
                issue_chunks(scalar, "scalar")
                scalar.wait_ge(vec_done, 2)
                scalar.dma_start(
                    out=sums_out[:, 256:512], in_=out_t[:, 256:512]
                ).then_inc(dma_out, 16)
                scalar.wait_ge(dma_out, 32)

            @block.gpsimd
            def _(gp: bass.BassEngine):
                issue_chunks(gp, "gpsimd")

            @block.tensor
            def _(tensor: bass.BassEngine):
                # Ramp the PE clock (HAM gate) with junk matmuls while the
                # first DMA chunks are in flight.
                for _ in range(NWARM):
                    nc.tensor.matmul(
                        psum_warm[:],
                        warm_t[:, 0:128],
                        warm_t[:, 128:640],
                        start=True,
                        stop=True,
                    )
                first_pr = CHUNK_OFF[PE_ORDER[0]] // 2
                last_pr = CHUNK_OFF[PE_ORDER[-1]] // 2
                for i, j in enumerate(PE_ORDER):
                    if i > 0:
                        # filler junk matmul: keeps the HAM activity window
                        # fed while waiting out the next chunk's semaphore,
                        # so the PE clock stays at 2.4 GHz.
                        nc.tensor.matmul(
                            psum_warm[:],
                            warm_t[:, 0:128],
                            warm_t[:, 128:640],
                            start=True,
                            stop=True,
                        )
                    tensor.wait_ge(oh_gen, j + 1)
                    tensor.wait_ge(x_sems[j], 16)
                    for pr in range(CHUNK_OFF[j] // 2, CHUNK_OFF[j + 1] // 2):
                        lhsT = oh_t[:, pr * 2 * C : (pr + 1) * 2 * C].rearrange(
                            "p (r c) -> p r c", r=2
                        )
                        rhs = x_t[
                            :, CW + pr * 2 * BWB : CW + (pr + 1) * 2 * BWB
                        ].rearrange("p (r w) -> p r w", r=2)
                        nc.tensor.matmul(
                            psum_acc[:],
                            lhsT,
                            rhs,
                            start=(pr == first_pr),
                            stop=(pr == last_pr + CHUNKS[PE_ORDER[-1]] // 2 - 1),
                            perf_mode=mybir.MatmulPerfMode.DoubleRow,
                        )
                tensor.drain().then_inc(pe_done, 1)

            @block.vector
            def _(vector: bass.BassEngine):
                # One-hot generation: oh[p, b*C + c] = (c == targets[b*128+p])
                # as fp8, one tensor_tensor per x-chunk via broadcast APs.
                # consts ride in front of chunk 0; view them as int8.
                vector.wait_ge(x_sems[0], 16)
                consts_ap = x_t[:, 0:CW].bitcast(I8)
                iota_ap = consts_ap[:, 0:C].unsqueeze(1)
                for j in range(NCHUNK):
                    b0, b1 = CHUNK_OFF[j], CHUNK_OFF[j + 1]
                    nb = b1 - b0
                    o3 = oh_t[:, b0 * C : b1 * C].rearrange("p (j c) -> p j c", j=nb)
                    io3 = iota_ap.broadcast_to([P, nb, C])
                    tg3 = (
                        consts_ap[:, C + b0 : C + b1]
                        .unsqueeze(2)
                        .broadcast_to([P, nb, C])
                    )
                    nc.vector.tensor_tensor(
                        o3, io3, tg3, mybir.AluOpType.is_equal
                    ).then_inc(oh_gen, 1)
                vector.wait_ge(pe_done, 1)
                nc.vector.tensor_copy(out_t[:, 0:256], psum_acc[:, 0:256]).then_inc(
                    vec_done, 1
                )
                nc.vector.tensor_copy(out_t[:, 256:512], psum_acc[:, 256:512]).then_inc(
                    vec_done, 1
                )

    return nc


def _get_program() -> bass.Bass:
    global _PROGRAM
    if _PROGRAM is None:
        _PROGRAM = _build_program()
    return _PROGRAM


def _make_in_maps(modal1, modal2, targets):
    x1 = np.asarray(modal1, dtype=np.float32)
    x2 = np.asarray(modal2, dtype=np.float32)
    targets = np.asarray(targets)

    x1q = x1.astype(NPFP8)
    x2q = x2.astype(NPFP8)

    tgt_pb = targets.reshape(NB, P).T.astype(np.int8)  # [p, b] = targets[b*128+p]
    iota = np.tile(np.arange(C, dtype=np.int8), (P, 1))
    consts = np.concatenate([iota, tgt_pb], axis=1).view(NPFP8)  # int8 bits

    in_maps = []
    for k in range(NCORES):
        sl = slice(k * DCHUNK, (k + 1) * DCHUNK)
        # [NB, P, 512] -> [P, NB*512]: block b cols = [x1 d-chunk | x2 d-chunk]
        blk = np.concatenate(
            [x1q[:, sl].reshape(NB, P, DCHUNK), x2q[:, sl].reshape(NB, P, DCHUNK)],
            axis=2,
        )
        xpk = blk.transpose(1, 0, 2).reshape(P, NB * BWB)
        in_maps.append(
            {"x": np.ascontiguousarray(np.concatenate([consts, xpk], axis=1))}
        )
    return in_maps


def _finish_on_host(sums_list, targets):
    """Recombine per-core segment sums, form class Grams, do class-pair loss."""
    P1 = np.zeros((C, C), np.float64)
    P2 = np.zeros((C, C), np.float64)
    P3 = np.zeros((C, C), np.float64)
    for s in sums_list:
        s = np.asarray(s, np.float64)
        R = s[:, 0:256]                      # [class, d-chunk]
        T = s[:, 256:512]
        P1 += R @ R.T
        P2 += T @ T.T
        P3 += R @ T.T

    n = np.bincount(targets, minlength=C).astype(np.float64)
    u = 1.0 / np.maximum(n, 1.0)

    S_CC = P1 + P2 + P3 + P3.T  # (R+T)(R+T)^T
    uu = np.outer(u, u)
    A1 = 0.5 * uu * (P1 + P3)    # meanR . ctr
    A2 = 0.5 * uu * (P2 + P3.T)  # meanT . ctr
    nR = u * u * np.diag(P1)
    nT = u * u * np.diag(P2)
    nCtr = 0.25 * u * u * np.diag(S_CC)

    W = np.outer(n, n)
    eye = np.eye(C)
    total = 0.0
    for A, nrm in ((A1, nR), (A2, nT)):
        sq = np.maximum(nrm[:, None] + nCtr[None, :] - 2.0 * A, 1e-12)
        d = np.sqrt(sq)
        dd = np.sqrt(d + 1e-10)
        term = eye * sq + (1.0 - eye) * np.maximum(MARGIN - dd, 0.0) ** 2
        total += (W * term).sum() / (float(N) * float(N))
    return np.asarray(total, dtype=np.float32)


def kernel(modal1_inputs, modal2_inputs, targets):
    nc = _get_program()
    in_maps = _make_in_maps(modal1_inputs, modal2_inputs, targets)
    res = run_bass_kernel_spmd(nc, in_maps, list(range(NCORES)))
    sums_list = [res.results[k]["sums"] for k in range(NCORES)]
    return _finish_on_host(sums_list, np.asarray(targets))


# revision 13
# speedup vs baseline: 1.1606x; 1.1606x over previous
"""Trainium2 Bass kernel for nn_Cross_modal_ContrastiveLoss6.

Math: the reference loss only depends on per-class means of the two
modalities (every entry of the N x N distance matrix is determined by the
class pair), so the whole computation reduces to:

  1. raw per-class segment sums R[c,d], T[c,d]  (memory-bound pass over x)
  2. the three 128x128 class Gram matrices P1 = R R^T, P2 = T T^T, P3 = R T^T
  3. tiny 128x128 class-pair loss math with the class counts

Device strategy (8 cores, feature/d-sharded so no cross-core collective is
needed): core k takes columns [256k, 256k+256) of both modal tensors and
computes the full-N segment sums for its d-chunk with one-hot matmuls on
the PE.  The x data is quantized host-side to fp8 e4m3 (measured end-to-end
loss rel-err ~6e-4, well inside the 2e-2 gate) which makes the HBM read
2.1 MiB per core.  Sample blocks are processed in PAIRS with fp8 DoubleRow
matmuls ([128, 2, 128] one-hot stationary x [128, 2, 512] moving, 2 MACs
per cell per cycle) so the PE keeps up with the DMA stream even while the
power manager caps it at half clock during heavy DMA.  One-hot stationaries
are generated on the otherwise idle DVE from an int8 iota row and packed
targets (prepended to x chunk 0, bit-cast in SBUF) via a broadcast-AP
is_equal.  The stream is spread over all three DMA-capable queues
(sync/scalar HWDGE + gpsimd SWDGE).  The segment sums return as bf16; the
host forms the three Grams and does the count scaling + sqrt/relu/weighted
mean (<0.1% of the FLOPs) in float64.
"""

import numpy as np
import ml_dtypes

import concourse.bass as bass
import concourse.mybir as mybir
from concourse.bass_utils import run_bass_kernel_spmd

N = 4096
D = 2048
C = 128
MARGIN = 0.5
NCORES = 8
DCHUNK = D // NCORES          # 256 feature columns per core
P = 128                       # partitions / sample-block size
NB = N // P                   # 32 sample blocks
BWB = 512                     # fp8 bytes per partition per block (x1 256 | x2 256)
CW = C + NB                   # consts bytes per partition (iota row + targets)
# Chunks (in 128-sample blocks; all even so DoubleRow pairs never straddle),
# round-robin over the sync / scalar / gpsimd queues.  Entries on one queue
# stream back-to-back; each entry's semaphore lags its last byte by ~1.4us,
# so chunks are kept moderate for smooth PE gating, and the consts ride in
# front of chunk 0 (no separate tiny transfer).
CHUNKS = [2, 4, 4, 6, 6, 4, 4, 2]
QUEUE = ["sync", "scalar", "sync", "scalar", "sync", "scalar", "sync", "scalar"]
# PE consumes chunks in expected-semaphore-arrival order (PSUM accumulation
# is additive, so any fixed order is valid).
PE_ORDER = [0, 1, 2, 3, 4, 6, 5, 7]
NCHUNK = len(CHUNKS)
CHUNK_OFF = [sum(CHUNKS[:i]) for i in range(NCHUNK + 1)]
NWARM = 7                     # junk matmuls to ramp the PE clock (HAM gate)

F32 = mybir.dt.float32
BF16 = mybir.dt.bfloat16
I8 = mybir.dt.int8
FP8 = mybir.dt.float8e4
NPBF16 = ml_dtypes.bfloat16
NPFP8 = ml_dtypes.float8_e4m3

_PROGRAM = None


def _build_program() -> bass.Bass:
    nc = bass.Bass()

    # x[:, 0:160]   = consts (int8 bits in the fp8 container): iota row
    #                 (iota[p, c] = c) then targets tgt[p, b] = targets[b*128+p].
    # x[:, 160 + b*512 : 160 + (b+1)*512] = fp8 [x1 | x2] for sample block b.
    x_in = nc.declare_dram_parameter("x", [P, CW + NB * BWB], FP8, isOutput=False)
    # sums[:, 0:256] = R segment sums for this core's d-chunk, [:, 256:512] = T
    sums_out = nc.declare_dram_parameter("sums", [P, 512], BF16, isOutput=True)

    import contextlib

    with contextlib.ExitStack() as stack:
        x_t = stack.enter_context(nc.sbuf_tensor([P, CW + NB * BWB], FP8))
        oh_t = stack.enter_context(nc.sbuf_tensor([P, NB * C], FP8))
        warm_t = stack.enter_context(nc.sbuf_tensor([P, 640], FP8))
        out_t = stack.enter_context(nc.sbuf_tensor([P, 512], BF16))
        psum_acc = stack.enter_context(nc.psum_tensor([P, 512], F32))
        psum_warm = stack.enter_context(nc.psum_tensor([P, 512], F32))

        def sem(name):
            return stack.enter_context(nc.semaphore(name))

        oh_gen = sem("oh_gen")
        x_sems = [sem(f"x_dma_{j}") for j in range(NCHUNK)]
        pe_done = sem("pe_done")
        vec_done = sem("vec_done")
        gp_done = sem("gp_done")
        dma_out = sem("dma_out")

        # Raw-bass semaphores are NOT cleared by the framework preamble; zero
        # them on gpsimd before the Block.  The Block-entry rendezvous keeps
        # every other engine from touching them until the clear has run.
        all_sems = [oh_gen] + x_sems + [pe_done, vec_done, gp_done, dma_out]
        nums = sorted(h.num for h in all_sems)
        assert nums == list(range(nums[0], nums[0] + len(nums))), nums
        nc.gpsimd.sem_clear(range(nums[0], nums[-1] + 1))

        def x_slice(j):
            a = CHUNK_OFF[j] * BWB + (0 if j == 0 else CW)
            b = CHUNK_OFF[j + 1] * BWB + CW
            return slice(a, b)

        def issue_chunks(eng, name):
            for j in range(NCHUNK):
                if QUEUE[j] != name:
                    continue
                sl = x_slice(j)
                eng.dma_start(out=x_t[:, sl], in_=x_in[:, sl]).then_inc(
                    x_sems[j], 16
                )

        with nc.Block(no_gpsimd_drain=True) as block:

            @block.sync
            def _(sync: bass.BassEngine):
                issue_chunks(sync, "sync")
                sync.wait_ge(vec_done, 1)
                sync.dma_start(
                    out=sums_out[:, 0:256], in_=out_t[:, 0:256]
                ).then_inc(dma_out, 16)
                sync.wait_ge(dma_out, 32)

            @block.scalar
            def _(scalar: bass.BassEngine):
                issue_chunks(scalar, "scalar")
                scalar.wait_ge(vec_done, 2)
                scalar.dma_start(
                    out=sums_out[:, 256:512], in_=out_t[:, 256:512]
                ).then_inc(dma_out, 16)
                scalar.wait_ge(dma_out, 32)

            @block.gpsimd
            def _(gp: bass.BassEngine):
                issue_chunks(gp, "gpsimd")

            @block.tensor
            def _(tensor: bass.BassEngine):
                # Ramp the PE clock (HAM gate) with junk matmuls while the
                # first DMA chunks are in flight.
                for _ in range(NWARM):
                    nc.tensor.matmul(
                        psum_warm[:],
                        warm_t[:, 0:128],
                        warm_t[:, 128:640],
                        start=True,
                        stop=True,
                    )
                first_pr = CHUNK_OFF[PE_ORDER[0]] // 2
                last_pr = CHUNK_OFF[PE_ORDER[-1]] // 2
                for i, j in enumerate(PE_ORDER):
                    if i > 0:
                        # filler junk matmul: keeps the HAM activity window
                        # fed while waiting out the next chunk's semaphore,
                        # so the PE clock stays at 2.4 GHz.
                        nc.tensor.matmul(
                            psum_warm[:],
                            warm_t[:, 0:128],
                            warm_t[:, 128:640],
                            start=True,
                            stop=True,
                        )
                    tensor.wait_ge(oh_gen, j + 1)
                    tensor.wait_ge(x_sems[j], 16)
                    for pr in range(CHUNK_OFF[j] // 2, CHUNK_OFF[j + 1] // 2):
                        lhsT = oh_t[:, pr * 2 * C : (pr + 1) * 2 * C].rearrange(
                            "p (r c) -> p r c", r=2
                        )
                        rhs = x_t[
                            :, CW + pr * 2 * BWB : CW + (pr + 1) * 2 * BWB
                        ].rearrange("p (r w) -> p r w", r=2)
                        nc.tensor.matmul(
                            psum_acc[:],
                            lhsT,
                            rhs,
                            start=(pr == first_pr),
                            stop=(pr == last_pr + CHUNKS[PE_ORDER[-1]] // 2 - 1),
                            perf_mode=mybir.MatmulPerfMode.DoubleRow,
                        )
                tensor.drain().then_inc(pe_done, 1)

            @block.vector
            def _(vector: bass.BassEngine):
                # One-hot generation: oh[p, b*C + c] = (c == targets[b*128+p])
                # as fp8, one tensor_tensor per x-chunk via broadcast APs.
                # consts ride in front of chunk 0; view them as int8.
                vector.wait_ge(x_sems[0], 16)
                consts_ap = x_t[:, 0:CW].bitcast(I8)
                iota_ap = consts_ap[:, 0:C].unsqueeze(1)
                for j in range(NCHUNK):
                    b0, b1 = CHUNK_OFF[j], CHUNK_OFF[j + 1]
                    nb = b1 - b0
                    o3 = oh_t[:, b0 * C : b1 * C].rearrange("p (j c) -> p j c", j=nb)
                    io3 = iota_ap.broadcast_to([P, nb, C])
                    tg3 = (
                        consts_ap[:, C + b0 : C + b1]
                        .unsqueeze(2)
                        .broadcast_to([P, nb, C])
                    )
                    nc.vector.tensor_tensor(
                        o3, io3, tg3, mybir.AluOpType.is_equal
                    ).then_inc(oh_gen, 1)
                vector.wait_ge(pe_done, 1)
                nc.vector.tensor_copy(out_t[:, 0:256], psum_acc[:, 0:256]).then_inc(
                    vec_done, 1
                )
                nc.vector.tensor_copy(out_t[:, 256:512], psum_acc[:, 256:512]).then_inc(
                    vec_done, 1
                )

    return nc


def _get_program() -> bass.Bass:
    global _PROGRAM
    if _PROGRAM is None:
        _PROGRAM = _build_program()
    return _PROGRAM


def _make_in_maps(modal1, modal2, targets):
    x1 = np.asarray(modal1, dtype=np.float32)
    x2 = np.asarray(modal2, dtype=np.float32)
    targets = np.asarray(targets)

    x1q = x1.astype(NPFP8)
    x2q = x2.astype(NPFP8)

    tgt_pb = targets.reshape(NB, P).T.astype(np.int8)  # [p, b] = targets[b*128+p]
    iota = np.tile(np.arange(C, dtype=np.int8), (P, 1))
    consts = np.concatenate([iota, tgt_pb], axis=1).view(NPFP8)  # int8 bits

    in_maps = []
    for k in range(NCORES):
        sl = slice(k * DCHUNK, (k + 1) * DCHUNK)
        # [NB, P, 512] -> [P, NB*512]: block b cols = [x1 d-chunk | x2 d-chunk]
        blk = np.concatenate(
            [x1q[:, sl].reshape(NB, P, DCHUNK), x2q[:, sl].reshape(NB, P, DCHUNK)],
            axis=2,
        )
        xpk = blk.transpose(1, 0, 2).reshape(P, NB * BWB)
        in_maps.append(
            {"x": np.ascontiguousarray(np.concatenate([consts, xpk], axis=1))}
        )
    return in_maps


def _finish_on_host(sums_list, targets):
    """Recombine per-core segment sums, form class Grams, do class-pair loss."""
    P1 = np.zeros((C, C), np.float64)
    P2 = np.zeros((C, C), np.float64)
    P3 = np.zeros((C, C), np.float64)
    for s in sums_list:
        s = np.asarray(s, np.float64)
        R = s[:, 0:256]                      # [class, d-chunk]
        T = s[:, 256:512]
        P1 += R @ R.T
        P2 += T @ T.T
        P3 += R @ T.T

    n = np.bincount(targets, minlength=C).astype(np.float64)
    u = 1.0 / np.maximum(n, 1.0)

    S_CC = P1 + P2 + P3 + P3.T  # (R+T)(R+T)^T
    uu = np.outer(u, u)
    A1 = 0.5 * uu * (P1 + P3)    # meanR . ctr
    A2 = 0.5 * uu * (P2 + P3.T)  # meanT . ctr
    nR = u * u * np.diag(P1)
    nT = u * u * np.diag(P2)
    nCtr = 0.25 * u * u * np.diag(S_CC)

    W = np.outer(n, n)
    eye = np.eye(C)
    total = 0.0
    for A, nrm in ((A1, nR), (A2, nT)):
        sq = np.maximum(nrm[:, None] + nCtr[None, :] - 2.0 * A, 1e-12)
        d = np.sqrt(sq)
        dd = np.sqrt(d + 1e-10)
        term = eye * sq + (1.0 - eye) * np.maximum(MARGIN - dd, 0.0) ** 2
        total += (W * term).sum() / (float(N) * float(N))
    return np.asarray(total, dtype=np.float32)


def kernel(modal1_inputs, modal2_inputs, targets):
    nc = _get_program()
    in_maps = _make_in_maps(modal1_inputs, modal2_inputs, targets)
    res = run_bass_kernel_spmd(nc, in_maps, list(range(NCORES)))
    sums_list = [res.results[k]["sums"] for k in range(NCORES)]
    return _finish_on_host(sums_list, np.asarray(targets))
